# revision 1
# baseline (speedup 1.0000x reference)
"""Trainium2 Bass kernel for the BiRNN cross-entropy-loss problem.

Strategy (data-parallel over batch, 8 NeuronCores, 16 batch rows each):
  One fused on-device loop walks the forward chain (f_i) and the backward
  chain (b_{S-1-i}) together.  Per iteration, per direction: two PSUM
  matmuls (Wx x + Wh h) and one fused tanh+bias activation.  First-half
  states are parked in SBUF slab tiles (8 timesteps x 16 batch = 128
  cols); in the second half, every 8 iterations two timestep-slabs (one
  from each end of the sequence) become complete and are immediately
  projected (cat(f,b) @ Wo.T + bo via 3 PSUM matmuls), exponentiated
  (ACT), and reduced (DVE) into per-(t,b) softmax sums and
  target-weighted logit sums.  The tiny log()/final reduction runs on
  host over the 2x[128, 256] per-core outputs.

Layouts per core c (p = 16*t_in_slab + b_local):
  xT  [64, S*16]      xT[i, 16t+b] = inps[t, 16c+b, i]
  tgt [128, 64*S/8]   tgt[p, 64j+i] = targets[8j+t, 16c+b, i]
  outputs ssum/t1 [128, S/8]:  col j = slab j, row p as above.
"""
import numpy as np

S = 2048
BATCH = 128
H = 128
I = 64
B = 16
N_CORES = 8

_CACHE = {}


def _build_nc():
    import concourse.bacc as bacc
    import concourse.tile as tile
    from concourse import mybir

    F32 = mybir.dt.float32
    AF = mybir.ActivationFunctionType
    ALU = mybir.AluOpType
    AX = mybir.AxisListType

    half = S // 2
    nslab = S // 8
    sl_half = nslab // 2
    CH = 64
    CHT = 8

    nc = bacc.Bacc("TRN2", target_bir_lowering=False, debug=False, num_devices=1)
    xT_d = nc.dram_tensor("xT", [I, S * B], F32, kind="ExternalInput").ap()
    tgt_d = nc.dram_tensor("tgt", [128, I * nslab], F32, kind="ExternalInput").ap()
    wxT_d = nc.dram_tensor("wxT", [I, H], F32, kind="ExternalInput").ap()
    whT_d = nc.dram_tensor("whT", [H, H], F32, kind="ExternalInput").ap()
    bf_d = nc.dram_tensor("bf", [H, 1], F32, kind="ExternalInput").ap()
    woT_d = nc.dram_tensor("woT", [2 * H, I], F32, kind="ExternalInput").ap()
    bo_d = nc.dram_tensor("bo", [1, I], F32, kind="ExternalInput").ap()
    ssum_d = nc.dram_tensor("ssum", [128, nslab], F32, kind="ExternalOutput").ap()
    t1_d = nc.dram_tensor("t1", [128, nslab], F32, kind="ExternalOutput").ap()

    with tile.TileContext(nc) as tc:
        with (
            tc.tile_pool(name="const", bufs=1) as cpool,
            tc.tile_pool(name="fring", bufs=sl_half) as fpool,
            tc.tile_pool(name="bring", bufs=sl_half) as bpool,
            tc.tile_pool(name="fstag", bufs=2) as fspool,
            tc.tile_pool(name="bstag", bufs=2) as bspool,
            tc.tile_pool(name="xf", bufs=2) as xfpool,
            tc.tile_pool(name="xb", bufs=2) as xbpool,
            tc.tile_pool(name="tg", bufs=4) as tgpool,
            tc.tile_pool(name="scr", bufs=2) as scrpool,
            tc.tile_pool(name="res", bufs=1) as rpool,
            tc.tile_pool(name="pf", bufs=2, space="PSUM") as pfpool,
            tc.tile_pool(name="pb", bufs=2, space="PSUM") as pbpool,
            tc.tile_pool(name="pp", bufs=2, space="PSUM") as pppool,
        ):
            wx = cpool.tile([I, H], F32, tag="wx")
            nc.sync.dma_start(wx[:], wxT_d[:])
            wh = cpool.tile([H, H], F32, tag="wh")
            nc.sync.dma_start(wh[:], whT_d[:])
            bf = cpool.tile([H, 1], F32, tag="bf")
            nc.sync.dma_start(bf[:], bf_d[:])
            wo_top_t = cpool.tile([H, I], F32, tag="woTa")
            nc.sync.dma_start(wo_top_t[:], woT_d[0:H, :])
            wo_bot_t = cpool.tile([H, I], F32, tag="woTb")
            nc.sync.dma_start(wo_bot_t[:], woT_d[H:2 * H, :])
            bo = cpool.tile([1, I], F32, tag="bo")
            nc.sync.dma_start(bo[:], bo_d[:])
            ones1 = cpool.tile([1, H], F32, tag="ones1")
            nc.vector.memset(ones1[:], 1.0)
            wo_top = wo_top_t[:]
            wo_bot = wo_bot_t[:]

            ssum_all = rpool.tile([128, nslab], F32, tag="ssum")
            t1_all = rpool.tile([128, nslab], F32, tag="t1")

            f_tiles = [fpool.tile([128, 128], F32, tag="f", name=f"fring{j}")
                       for j in range(sl_half)]
            b_tiles = [bpool.tile([128, 128], F32, tag="b", name=f"bring{j}")
                       for j in range(sl_half)]

            xf_tiles, xb_tiles, tg_tiles = {}, {}, {}

            def load_x_chunk(k):
                if k < S // CH:
                    t = xfpool.tile([I, CH * B], F32, tag="xfc", name=f"xf{k}")
                    nc.sync.dma_start(t[:], xT_d[:, CH * B * k: CH * B * (k + 1)])
                    xf_tiles[k] = t
                    t2 = xbpool.tile([I, CH * B], F32, tag="xbc", name=f"xb{k}")
                    lo = B * (S - CH * (k + 1))
                    nc.sync.dma_start(t2[:], xT_d[:, lo: lo + CH * B])
                    xb_tiles[k] = t2

            def load_tgt_chunk(g):
                th = tgpool.tile([128, I * CHT], F32, tag="tgc", name=f"tgh{g}")
                j0 = sl_half + CHT * g
                nc.sync.dma_start(th[:], tgt_d[:, I * j0: I * (j0 + CHT)])
                tg_tiles[("h", g)] = th
                tl = tgpool.tile([128, I * CHT], F32, tag="tgc", name=f"tgl{g}")
                j1 = sl_half - CHT * (g + 1)
                nc.sync.dma_start(tl[:], tgt_d[:, I * j1: I * (j1 + CHT)])
                tg_tiles[("l", g)] = tl

            load_x_chunk(0)
            prev_f = prev_b = fs_cur = bs_cur = None
            n_tgt_chunks = sl_half // CHT

            for i in range(S):
                if i % CH == 0:
                    load_x_chunk(i // CH + 1)
                if i == half:
                    load_tgt_chunk(0)
                    if n_tgt_chunks > 1:
                        load_tgt_chunk(1)
                elif i > half and (i - half) % (8 * CHT) == 0:
                    g_next = (i - half) // (8 * CHT) + 1
                    if g_next < n_tgt_chunks:
                        load_tgt_chunk(g_next)

                k = i // CH
                lf = (i % CH) * B
                s_b = S - 1 - i
                lb = (s_b - (S - CH * (k + 1))) * B

                pf = pfpool.tile([128, B], F32, tag="pf")
                pb = pbpool.tile([128, B], F32, tag="pb")
                nc.tensor.matmul(pf[:], wx[:], xf_tiles[k][:, lf:lf + B],
                                 start=True, stop=(i == 0))
                if i > 0:
                    nc.tensor.matmul(pf[:], wh[:], prev_f, start=False, stop=True)
                nc.tensor.matmul(pb[:], wx[:], xb_tiles[k][:, lb:lb + B],
                                 start=True, stop=(i == 0))
                if i > 0:
                    nc.tensor.matmul(pb[:], wh[:], prev_b, start=False, stop=True)

                if i < half:
                    f_dst = f_tiles[i // 8][:, (i % 8) * B:(i % 8) * B + B]
                    b_dst = b_tiles[(s_b - half) // 8][:, (s_b % 8) * B:(s_b % 8) * B + B]
                else:
                    if i % 8 == 0:
                        fs_cur = fspool.tile([128, 128], F32, tag="fs")
                        bs_cur = bspool.tile([128, 128], F32, tag="bs")
                    f_dst = fs_cur[:, (i % 8) * B:(i % 8) * B + B]
                    b_dst = bs_cur[:, (s_b % 8) * B:(s_b % 8) * B + B]
                nc.scalar.activation(f_dst, pf[:], AF.Tanh, bias=bf[:, 0:1])
                nc.scalar.activation(b_dst, pb[:], AF.Tanh, bias=bf[:, 0:1])
                prev_f, prev_b = f_dst, b_dst

                if i >= half and i % 8 == 7:
                    j_hi = i // 8
                    j_lo = (S - 1 - i) // 8
                    pp_hi = pppool.tile([128, I], F32, tag="pp")
                    nc.tensor.matmul(pp_hi[:], fs_cur[:], wo_top, start=True, stop=False)
                    nc.tensor.matmul(pp_hi[:], b_tiles[j_hi - sl_half][:], wo_bot,
                                     start=False, stop=False)
                    nc.tensor.matmul(pp_hi[:], ones1[:], bo[:], start=False, stop=True)
                    pp_lo = pppool.tile([128, I], F32, tag="pp")
                    nc.tensor.matmul(pp_lo[:], f_tiles[j_lo][:], wo_top,
                                     start=True, stop=False)
                    nc.tensor.matmul(pp_lo[:], bs_cur[:], wo_bot, start=False, stop=False)
                    nc.tensor.matmul(pp_lo[:], ones1[:], bo[:], start=False, stop=True)
                    g = (i - half) // (8 * CHT)
                    for which, j, pp_x in (("h", j_hi, pp_hi), ("l", j_lo, pp_lo)):
                        if which == "h":
                            loc = (j - sl_half) % CHT
                        else:
                            g = (sl_half - 1 - j) // CHT
                            loc = j - (sl_half - CHT * (g + 1))
                        tslab = tg_tiles[(which, g)][:, I * loc: I * (loc + 1)]
                        e_scr = scrpool.tile([128, I], F32, tag="escr")
                        nc.scalar.activation(e_scr[:], pp_x[:], AF.Exp)
                        nc.vector.reduce_sum(ssum_all[:, j:j + 1], e_scr[:], axis=AX.X)
                        p_scr = scrpool.tile([128, I], F32, tag="pscr")
                        nc.vector.scalar_tensor_tensor(
                            p_scr[:], tslab, 1.0, pp_x[:],
                            op0=ALU.mult, op1=ALU.mult,
                            accum_out=t1_all[:, j:j + 1])

            nc.sync.dma_start(ssum_d[:], ssum_all[:])
            nc.sync.dma_start(t1_d[:], t1_all[:])

    nc.compile()
    return nc


def _get_runner():
    if "runner" in _CACHE:
        return _CACHE["runner"]
    import jax
    from jax.sharding import Mesh, PartitionSpec
    from jax.experimental.shard_map import shard_map
    import concourse.mybir as mybir
    from concourse.bass2jax import (_bass_exec_p, install_neuronx_cc_hook,
                                    partition_id_tensor)

    nc = _build_nc()
    install_neuronx_cc_hook()

    partition_name = (nc.partition_id_tensor.name
                      if nc.partition_id_tensor else None)
    in_names, out_names, out_avals, zero_outs = [], [], [], []
    for alloc in nc.m.functions[0].allocations:
        if not isinstance(alloc, mybir.MemoryLocationSet):
            continue
        name = alloc.memorylocations[0].name
        if alloc.kind == "ExternalInput":
            if name != partition_name:
                in_names.append(name)
        elif alloc.kind == "ExternalOutput":
            out_names.append(name)
            shape = tuple(alloc.tensor_shape)
            dtype = mybir.dt.np(alloc.dtype)
            out_avals.append(jax.core.ShapedArray(shape, dtype))
            zero_outs.append(np.zeros(shape, dtype))
    n_params = len(in_names)
    n_outs = len(out_avals)
    all_in_names = list(in_names) + list(out_names)
    if partition_name is not None:
        all_in_names.append(partition_name)
    donate = tuple(range(n_params, n_params + n_outs))

    def _body(*args):
        operands = list(args)
        if partition_name is not None:
            operands.append(partition_id_tensor())
        outs = _bass_exec_p.bind(
            *operands,
            out_avals=tuple(out_avals),
            in_names=tuple(all_in_names),
            out_names=tuple(out_names),
            lowering_input_output_aliases=(),
            sim_require_finite=True,
            sim_require_nnan=True,
            nc=nc,
        )
        return tuple(outs)

    devices = jax.devices()[:N_CORES]
    mesh = Mesh(np.asarray(devices), ("core",))
    in_specs = (PartitionSpec("core"),) * (n_params + n_outs)
    out_specs = (PartitionSpec("core"),) * len(out_names)
    fn = jax.jit(
        shard_map(_body, mesh=mesh, in_specs=in_specs, out_specs=out_specs,
                  check_rep=False),
        donate_argnums=donate, keep_unused=True,
    )

    def run(in_maps):
        per_core = [[np.asarray(m[name]) for name in in_names]
                    for m in in_maps]
        concat_in = [
            np.concatenate([per_core[c][k] for c in range(N_CORES)], axis=0)
            for k in range(n_params)
        ]
        zeros = [np.zeros((N_CORES * z.shape[0], *z.shape[1:]), z.dtype)
                 for z in zero_outs]
        out_arrs = fn(*concat_in, *zeros)
        return [
            {name: np.asarray(out_arrs[k]).reshape(N_CORES, *out_avals[k].shape)[c]
             for k, name in enumerate(out_names)}
            for c in range(N_CORES)
        ]

    _CACHE["runner"] = run
    return run


def _prep_core_inputs(inps, targets, Wf, bf, Wo, bo, core):
    b0 = core * B
    inps_c = np.ascontiguousarray(inps[:, b0:b0 + B, :])
    xT = np.ascontiguousarray(inps_c.transpose(2, 0, 1).reshape(I, S * B))
    t_c = targets[:, b0:b0 + B, :]
    tgt = np.ascontiguousarray(
        t_c.reshape(S // 8, 8 * B, I).transpose(1, 0, 2).reshape(8 * B, (S // 8) * I))
    return {
        "xT": xT.astype(np.float32),
        "tgt": tgt.astype(np.float32),
        "wxT": np.ascontiguousarray(Wf[:, :I].T).astype(np.float32),
        "whT": np.ascontiguousarray(Wf[:, I:].T).astype(np.float32),
        "bf": np.asarray(bf).reshape(H, 1).astype(np.float32),
        "woT": np.ascontiguousarray(Wo.T).astype(np.float32),
        "bo": np.asarray(bo).reshape(1, I).astype(np.float32),
    }


def kernel(inps, targets, Wf, bf, Wo, bo, batch_size=BATCH, seq_len=S, **_):
    inps = np.asarray(inps)
    targets = np.asarray(targets)
    Wf = np.asarray(Wf)
    bf = np.asarray(bf)
    Wo = np.asarray(Wo)
    bo = np.asarray(bo)

    run = _get_runner()
    in_maps = [_prep_core_inputs(inps, targets, Wf, bf, Wo, bo, c)
               for c in range(N_CORES)]
    results = run(in_maps)

    total = 0.0
    for c in range(N_CORES):
        ssum = results[c]["ssum"].astype(np.float64)
        t1 = results[c]["t1"].astype(np.float64)
        tgt = in_maps[c]["tgt"].astype(np.float64)
        tsum = tgt.reshape(128, S // 8, I).sum(axis=2)
        total += (t1 - np.log(ssum) * tsum).sum()
    return np.float32(-total / int(batch_size))
